# revision 27
# baseline (speedup 1.0000x reference)
"""Trainium2 Bass kernel for nn_NeRFMLPNetwork (StyleGAN-style modulated 1x1-conv MLP).

Network (per layer): s = affine(w_lat); y = conv1x1(x * s); y = y * rsqrt(demod) + b;
out = lrelu(y) * sqrt(2).  8 layers (60->128, then 7x 128->128), B=4, H*W=32768.

Strategy:
  - Data parallel over H*W: each of 8 cores handles 4096 spatial points (all batches).
  - The style path (affine styles s, demod d) is tiny (<0.01% of FLOPs) and is
    computed on the host, StyleGAN-eval style: the fully folded per-(layer,batch)
    weight  Wmod[c, o] = convT[c,o] * s[b,c] * d[b,o] * sqrt(2)  is shipped to the
    device (~2 MB), so the device does only: matmul + bias + lrelu + DMA.
  - Intermediate activations/weights ride as f32r (full-rate matmul at
    free-dim >= 256, ~2^-13 precision); the layer-0 input x + wm0 and the final
    output y ride as bf16 (halved DMA, same matmul speed, well within the
    rel-err budget).  Epilogue: out = prelu(psum + gcb, alpha=0.2) with
    gcb = sqrt(2)*cb, split ~6:5 between ScalarE (bias-only Prelu, 1147ns/1024
    cols) and VectorE (custom DVE op SCALE_BIAS_LRELU, 1379ns) so both
    PSUM-drain engines run flat out -- they are the roofline (PSUM reads are
    capped at 1 elem/cycle/lane/engine; GpSimd/DMA have no PSUM port).
  - Column pipeline: 2048-col sub-blocks striped in pairs so the PE queue
    interleaves two blocks per layer; per-1024-col PSUM groups (4 in flight);
    first x block rides the gpsimd DMA queue ahead of the weight stream; an
    8-matmul warm-up burst un-throttles the PE clock (HAM) during the DMA ramp.
"""

import numpy as np

import concourse.bacc as bacc
import concourse.mybir as mybir
import concourse.tile as tile
from concourse.bass_utils import run_bass_kernel_spmd

# ---------------------------------------------------------------------------
# Custom DVE op: out = max(z, z*imm2) with z = in0*s0 + s1   (leaky relu)
# ---------------------------------------------------------------------------
import concourse.dve_ops as dve_ops_mod
from concourse.dve_spec import Spec, Src0, C0, C1, C2, maxx, lower as _dve_lower
from concourse.dve_spec import _has_src1
from concourse.dve_uop import DveOpSpec


def _sbl_ref(in0, in1, s0, s1, imm2):
    z = in0.astype(np.float32) * s0 + s1
    return np.maximum(z, z * imm2)


_z = Src0 * C0 + C1
_SBL_SPEC = Spec(body=maxx(_z, _z * C2), reference=_sbl_ref)
SCALE_BIAS_LRELU = dve_ops_mod.DveOp(
    "SCALE_BIAS_LRELU", _SBL_SPEC, subdim=False, uops_sha={}
)
if "SCALE_BIAS_LRELU" not in dve_ops_mod._SUB_OPCODE_FOR_NAME:
    dve_ops_mod.OPS.append(SCALE_BIAS_LRELU)
    dve_ops_mod.CUSTOM_DVE_SPECS["SCALE_BIAS_LRELU"] = _SBL_SPEC
    dve_ops_mod._SUB_OPCODE_FOR_NAME["SCALE_BIAS_LRELU"] = (
        max(dve_ops_mod._SUB_OPCODE_FOR_NAME.values()) + 1
    )
for _ver in ("v3", "v4"):
    _s = DveOpSpec(
        name="SCALE_BIAS_LRELU",
        opcode=dve_ops_mod.get_dve_sub_opcode("SCALE_BIAS_LRELU"),
        uops=_dve_lower(_SBL_SPEC, ver=_ver),
        rd1_en=_has_src1(_SBL_SPEC),
    )
    SCALE_BIAS_LRELU.uops_sha[_ver] = _s.sha(_ver)

# ---------------------------------------------------------------------------
# Problem constants (hardcoded per spec)
# ---------------------------------------------------------------------------
B, CIN, H, W, HID, WDIM, NB = 4, 60, 64, 512, 128, 512, 8
HWTOT = H * W                    # 32768
N_CORES = 8
SHARD = HWTOT // N_CORES         # 4096 spatial points per core
INV_SQRT_WDIM = float(1.0 / np.sqrt(WDIM))
SQRT2 = float(np.sqrt(2.0))
EPS = 1e-8

F32 = mybir.dt.float32
F32R = mybir.dt.float32r
BF16 = mybir.dt.bfloat16

GROUP = 1024                     # psum group columns (2 banks)
SUB = 2048                       # x columns per sub-block (striped in pairs)
NT = GROUP // 512                # matmuls per psum group
ACT_SHARE = 6                    # of ACT_DEN groups go to ScalarE (rest VectorE)
ACT_DEN = 11
WARMUP = 8                       # dummy MMs at t=0 to un-throttle the PE clock

_COMPILED = None


def _build():
    nc = bacc.Bacc("TRN2", target_bir_lowering=False, debug=False,
                   num_devices=N_CORES)

    # declared f32r: raw f32 bits DMA directly; the PE rounds on read.
    x_d = nc.dram_tensor("x", [B, CIN, SHARD], BF16, kind="ExternalInput").ap()
    wm0_d = nc.dram_tensor("wm0", [CIN, B * HID], BF16, kind="ExternalInput").ap()
    wmr_d = nc.dram_tensor("wmr", [HID, NB - 1, B * HID], F32R,
                           kind="ExternalInput").ap()
    gcb_d = nc.dram_tensor("gcb", [HID, NB], F32, kind="ExternalInput").ap()
    y_d = nc.dram_tensor("y", [B, HID, SHARD], BF16, kind="ExternalOutput").ap()

    with tile.TileContext(nc) as tc:
        with (
            tc.tile_pool(name="big", bufs=8) as big,
            tc.tile_pool(name="x0p", bufs=4) as x0p,
            tc.tile_pool(name="wts", bufs=1) as wts,
            tc.tile_pool(name="xst", bufs=4) as xst,
            tc.tile_pool(name="ps", bufs=4, space="PSUM") as ps,
        ):
            # ---- DMA order: small params, first pair's x, then weights ----
            gcb = wts.tile([HID, NB], F32, tag="gcb")
            nc.sync.dma_start(gcb[:], gcb_d[:])
            ones = wts.tile([HID, 1], F32, tag="ones")
            nc.vector.memset(ones[:], 1.0)
            wm0 = wts.tile([CIN, B * HID], BF16, tag="wm0")
            nc.sync.dma_start(wm0[:, :HID], wm0_d[:, :HID])

            def load_sub(sub, eng=None):
                b, col0 = sub
                x0 = x0p.tile([CIN, SUB], BF16, tag="x0")
                bufA = big.tile([128, SUB], F32R, tag="xbuf")
                bufB = big.tile([128, SUB], F32R, tag="xbuf")
                (eng or nc.sync).dma_start(x0[:], x_d[b, :, col0:col0 + SUB])
                return x0, bufA, bufB

            subs = [(b, c0) for b in range(B) for c0 in range(0, SHARD, SUB)]
            pairs = [(subs[i], subs[i + 1]) for i in range(0, len(subs), 2)]
            bufs = {}
            # first pair rides the (otherwise idle) gpsimd trigger queue
            for s in pairs[0]:
                bufs[s] = load_sub(s, nc.gpsimd)
            nc.sync.dma_start(wm0[:, HID:], wm0_d[:, HID:])

            # layer weights stream in behind the first x pair
            wmr = wts.tile([HID, NB - 1, B * HID], F32R, tag="wmr")
            for _l in range(NB - 1):
                nc.sync.dma_start(wmr[:, _l, :], wmr_d[:, _l, :])

            # PE clock warm-up: HAM un-throttles (1.2 -> 2.4 GHz) only after
            # ~3.4us of continuous PE activity; burn that in during the x DMA
            # so every real matmul runs at full clock.
            wrm = wts.tile([128, 512], F32, tag="wrm")
            nc.vector.memset(wrm[:], 0.5)
            for _i in range(WARMUP):
                ptw = ps.tile([128, GROUP], F32, tag="ps")
                nc.tensor.matmul(ptw[:, :512], wrm[:, :128].bitcast(F32R),
                                 wrm[:].bitcast(F32R), start=True, stop=True)

            gcnt = 0
            for pi, pair in enumerate(pairs):
                if pi + 1 < len(pairs):
                    for s in pairs[pi + 1]:
                        bufs[s] = load_sub(s)
                for l in range(NB):
                    C = CIN if l == 0 else HID
                    wmod = wm0 if l == 0 else wmr[:, l - 1, :]
                    gc = gcb[:, l:l + 1]
                    last = l == NB - 1
                    for s in pair:
                        b, col0 = s
                        x0, bufA, bufB = bufs[s]
                        x_in = x0 if l == 0 else (bufA if l % 2 == 1 else bufB)
                        x_out = bufA if l % 2 == 0 else bufB
                        for g in range(SUB // GROUP):
                            pt = ps.tile([128, GROUP], F32, tag="ps")
                            c0 = g * GROUP
                            for t in range(NT):
                                nc.tensor.matmul(
                                    pt[:, t * 512:(t + 1) * 512],
                                    wmod[:C, b * HID:(b + 1) * HID],
                                    x_in[:C, c0 + t * 512:c0 + (t + 1) * 512],
                                    start=True, stop=True)
                            # epilogue: out = prelu(psum + gcb, 0.2)
                            if last:
                                ost = xst.tile([128, GROUP], BF16, tag="xout")
                                o_full = ost[:]
                            else:
                                o_full = x_out[:, c0:c0 + GROUP]
                            gi = 0 if (gcnt * ACT_SHARE) % ACT_DEN < ACT_SHARE \
                                else 1
                            gcnt += 1
                            if gi == 0:
                                nc.scalar.activation(
                                    o_full, pt[:],
                                    mybir.ActivationFunctionType.Prelu,
                                    bias=gc, alpha=0.2)
                            else:
                                nc.vector._custom_dve(
                                    SCALE_BIAS_LRELU,
                                    out=o_full, in0=pt[:],
                                    s0=ones[:, 0:1], s1=gc,
                                    imm2=0.2)
                            if last:
                                nc.gpsimd.dma_start(
                                    y_d[b, :, col0 + g * GROUP:
                                        col0 + (g + 1) * GROUP],
                                    ost[:])
                for s in pair:
                    del bufs[s]

    nc.compile()
    return nc


def _prep_inputs(pre_point_features, points_encoding, wp,
                 aff_w_in, aff_b_in, conv_w_in, conv_b_in,
                 aff_w, aff_b, conv_w, conv_b):
    """Host-side prep: layout of x + the (tiny) style path fully folded into
    per-(layer,batch) modulated-demodulated weights."""
    import ml_dtypes
    x = np.ascontiguousarray(np.asarray(points_encoding, np.float32)
                             .reshape(B, CIN, HWTOT)
                             .astype(ml_dtypes.bfloat16))
    wp = np.asarray(wp, np.float32)

    def fold(wl, aw, ab, cw):
        # wl [B,WDIM]; aw [C,WDIM]; ab [C]; cw [O,C] -> wmod [C, B*O]
        s = wl @ aw.T * INV_SQRT_WDIM + ab                      # [B, C]
        d = 1.0 / np.sqrt((s * s) @ (cw * cw).T + EPS)          # [B, O]
        wmod = (cw.T[None, :, :] * s[:, :, None] * d[:, None, :]
                * SQRT2)                                        # [B, C, O]
        return np.ascontiguousarray(
            wmod.transpose(1, 0, 2).reshape(wmod.shape[1], B * cw.shape[0]))

    aff_w_in = np.asarray(aff_w_in, np.float32)
    aff_b_in = np.asarray(aff_b_in, np.float32)
    conv_w_in = np.asarray(conv_w_in, np.float32)
    aff_w = np.asarray(aff_w, np.float32)
    aff_b = np.asarray(aff_b, np.float32)
    conv_w = np.asarray(conv_w, np.float32)

    wm0 = fold(wp[:, 0], aff_w_in, aff_b_in, conv_w_in)          # [CIN, B*HID]
    wm0 = wm0.astype(ml_dtypes.bfloat16)
    wmr = np.stack([fold(wp[:, 1 + i], aff_w[i], aff_b[i], conv_w[i])
                    for i in range(NB - 1)], axis=1)             # [HID,NB-1,B*HID]
    wmr = np.ascontiguousarray(wmr)

    gcb = np.empty((HID, NB), np.float32)
    gcb[:, 0] = SQRT2 * np.asarray(conv_b_in, np.float32)
    gcb[:, 1:] = SQRT2 * np.asarray(conv_b, np.float32).T

    shared = dict(wm0=wm0, wmr=wmr, gcb=gcb)
    in_maps = []
    for c in range(N_CORES):
        m = dict(shared)
        m["x"] = np.ascontiguousarray(x[:, :, c * SHARD:(c + 1) * SHARD])
        in_maps.append(m)
    return in_maps


def kernel(trace=False, **inputs):
    global _COMPILED
    if _COMPILED is None:
        _COMPILED = _build()
    nc = _COMPILED
    in_maps = _prep_inputs(**inputs)
    res = run_bass_kernel_spmd(nc, in_maps, core_ids=list(range(N_CORES)),
                               trace=trace)
    parts = [np.asarray(res.results[c]["y"]).astype(np.float32)
             for c in range(N_CORES)]
    out = np.concatenate(parts, axis=2).reshape(B, HID, H, W)
    if trace:
        kernel.last_result = res
    return out


# revision 28
# speedup vs baseline: 1.0140x; 1.0140x over previous
"""Trainium2 Bass kernel for nn_NeRFMLPNetwork (StyleGAN-style modulated 1x1-conv MLP).

Network (per layer): s = affine(w_lat); y = conv1x1(x * s); y = y * rsqrt(demod) + b;
out = lrelu(y) * sqrt(2).  8 layers (60->128, then 7x 128->128), B=4, H*W=32768.

Strategy:
  - Data parallel over H*W: each of 8 cores handles 4096 spatial points (all batches).
  - The style path (affine styles s, demod d) is tiny (<0.01% of FLOPs) and is
    computed on the host, StyleGAN-eval style: the fully folded per-(layer,batch)
    weight  Wmod[c, o] = convT[c,o] * s[b,c] * d[b,o] * sqrt(2)  is shipped to the
    device (~2 MB), so the device does only: matmul + bias + lrelu + DMA.
  - Intermediate activations/weights ride as f32r (full-rate matmul at
    free-dim >= 256, ~2^-13 precision); the layer-0 input x + wm0 and the final
    output y ride as bf16 (halved DMA, same matmul speed, well within the
    rel-err budget).  Epilogue: out = prelu(psum + gcb, alpha=0.2) with
    gcb = sqrt(2)*cb, split ~6:5 between ScalarE (bias-only Prelu, 1147ns/1024
    cols) and VectorE (custom DVE op SCALE_BIAS_LRELU, 1379ns) so both
    PSUM-drain engines run flat out -- they are the roofline (PSUM reads are
    capped at 1 elem/cycle/lane/engine; GpSimd/DMA have no PSUM port).
  - Column pipeline: 2048-col sub-blocks striped in pairs so the PE queue
    interleaves two blocks per layer; per-1024-col PSUM groups (4 in flight);
    first x block rides the gpsimd DMA queue ahead of the weight stream; an
    8-matmul warm-up burst un-throttles the PE clock (HAM) during the DMA ramp.
"""

import numpy as np

import concourse.bacc as bacc
import concourse.mybir as mybir
import concourse.tile as tile
from concourse.bass_utils import run_bass_kernel_spmd

# ---------------------------------------------------------------------------
# Custom DVE op: out = max(z, z*imm2) with z = in0*s0 + s1   (leaky relu)
# ---------------------------------------------------------------------------
import concourse.dve_ops as dve_ops_mod
from concourse.dve_spec import Spec, Src0, C0, C1, C2, maxx, lower as _dve_lower
from concourse.dve_spec import _has_src1
from concourse.dve_uop import DveOpSpec


def _sbl_ref(in0, in1, s0, s1, imm2):
    z = in0.astype(np.float32) * s0 + s1
    return np.maximum(z, z * imm2)


_z = Src0 * C0 + C1
_SBL_SPEC = Spec(body=maxx(_z, _z * C2), reference=_sbl_ref)
SCALE_BIAS_LRELU = dve_ops_mod.DveOp(
    "SCALE_BIAS_LRELU", _SBL_SPEC, subdim=False, uops_sha={}
)
if "SCALE_BIAS_LRELU" not in dve_ops_mod._SUB_OPCODE_FOR_NAME:
    dve_ops_mod.OPS.append(SCALE_BIAS_LRELU)
    dve_ops_mod.CUSTOM_DVE_SPECS["SCALE_BIAS_LRELU"] = _SBL_SPEC
    dve_ops_mod._SUB_OPCODE_FOR_NAME["SCALE_BIAS_LRELU"] = (
        max(dve_ops_mod._SUB_OPCODE_FOR_NAME.values()) + 1
    )
for _ver in ("v3", "v4"):
    _s = DveOpSpec(
        name="SCALE_BIAS_LRELU",
        opcode=dve_ops_mod.get_dve_sub_opcode("SCALE_BIAS_LRELU"),
        uops=_dve_lower(_SBL_SPEC, ver=_ver),
        rd1_en=_has_src1(_SBL_SPEC),
    )
    SCALE_BIAS_LRELU.uops_sha[_ver] = _s.sha(_ver)

# ---------------------------------------------------------------------------
# Problem constants (hardcoded per spec)
# ---------------------------------------------------------------------------
B, CIN, H, W, HID, WDIM, NB = 4, 60, 64, 512, 128, 512, 8
HWTOT = H * W                    # 32768
N_CORES = 8
SHARD = HWTOT // N_CORES         # 4096 spatial points per core
INV_SQRT_WDIM = float(1.0 / np.sqrt(WDIM))
SQRT2 = float(np.sqrt(2.0))
EPS = 1e-8

F32 = mybir.dt.float32
F32R = mybir.dt.float32r
BF16 = mybir.dt.bfloat16

GROUP = 1024                     # psum group columns (2 banks)
SUB = 2048                       # x columns per sub-block (striped in pairs)
NT = GROUP // 512                # matmuls per psum group
ACT_SHARE = 6                    # of ACT_DEN groups go to ScalarE (rest VectorE)
ACT_DEN = 11
WARMUP = 14                      # dummy MMs at t=0 to un-throttle the PE clock

_COMPILED = None


def _build():
    nc = bacc.Bacc("TRN2", target_bir_lowering=False, debug=False,
                   num_devices=N_CORES)

    # declared f32r: raw f32 bits DMA directly; the PE rounds on read.
    x_d = nc.dram_tensor("x", [B, CIN, SHARD], BF16, kind="ExternalInput").ap()
    wm0_d = nc.dram_tensor("wm0", [CIN, B * HID], BF16, kind="ExternalInput").ap()
    wmr_d = nc.dram_tensor("wmr", [HID, NB - 1, B * HID], F32R,
                           kind="ExternalInput").ap()
    gcb_d = nc.dram_tensor("gcb", [HID, NB], F32, kind="ExternalInput").ap()
    y_d = nc.dram_tensor("y", [B, HID, SHARD], BF16, kind="ExternalOutput").ap()

    with tile.TileContext(nc) as tc:
        with (
            tc.tile_pool(name="big", bufs=8) as big,
            tc.tile_pool(name="x0p", bufs=4) as x0p,
            tc.tile_pool(name="wts", bufs=1) as wts,
            tc.tile_pool(name="xst", bufs=4) as xst,
            tc.tile_pool(name="ps", bufs=4, space="PSUM") as ps,
        ):
            # ---- DMA order: layer-0 weight slice first, then params ----
            ones = wts.tile([HID, 1], F32, tag="ones")
            nc.vector.memset(ones[:], 1.0)
            wm0 = wts.tile([CIN, B * HID], BF16, tag="wm0")
            nc.sync.dma_start(wm0[:, :HID], wm0_d[:, :HID])
            gcb = wts.tile([HID, NB], F32, tag="gcb")
            nc.sync.dma_start(gcb[:], gcb_d[:])

            def load_sub(sub, eng=None):
                b, col0 = sub
                x0 = x0p.tile([CIN, SUB], BF16, tag="x0")
                bufA = big.tile([128, SUB], F32R, tag="xbuf")
                bufB = big.tile([128, SUB], F32R, tag="xbuf")
                (eng or nc.sync).dma_start(x0[:], x_d[b, :, col0:col0 + SUB])
                return x0, bufA, bufB

            subs = [(b, c0) for b in range(B) for c0 in range(0, SHARD, SUB)]
            pairs = [(subs[i], subs[i + 1]) for i in range(0, len(subs), 2)]
            bufs = {}
            # first pair rides the (otherwise idle) gpsimd trigger queue
            for s in pairs[0]:
                bufs[s] = load_sub(s, nc.gpsimd)
            nc.sync.dma_start(wm0[:, HID:], wm0_d[:, HID:])

            # layer weights stream in behind the first x pair
            wmr = wts.tile([HID, NB - 1, B * HID], F32R, tag="wmr")
            for _l in range(NB - 1):
                nc.sync.dma_start(wmr[:, _l, :], wmr_d[:, _l, :])

            # PE clock warm-up: HAM un-throttles (1.2 -> 2.4 GHz) only after
            # ~3.4us of continuous PE activity; burn that in during the x DMA
            # so every real matmul runs at full clock.
            wrm = wts.tile([128, 512], F32, tag="wrm")
            nc.vector.memset(wrm[:], 0.5)
            for _i in range(WARMUP):
                ptw = ps.tile([128, GROUP], F32, tag="ps")
                nc.tensor.matmul(ptw[:, :512], wrm[:, :128].bitcast(F32R),
                                 wrm[:].bitcast(F32R), start=True, stop=True)

            gcnt = 0
            for pi, pair in enumerate(pairs):
                if pi + 1 < len(pairs):
                    for s in pairs[pi + 1]:
                        bufs[s] = load_sub(s)
                for l in range(NB):
                    C = CIN if l == 0 else HID
                    wmod = wm0 if l == 0 else wmr[:, l - 1, :]
                    gc = gcb[:, l:l + 1]
                    last = l == NB - 1
                    for s in pair:
                        b, col0 = s
                        x0, bufA, bufB = bufs[s]
                        x_in = x0 if l == 0 else (bufA if l % 2 == 1 else bufB)
                        x_out = bufA if l % 2 == 0 else bufB
                        for g in range(SUB // GROUP):
                            pt = ps.tile([128, GROUP], F32, tag="ps")
                            c0 = g * GROUP
                            for t in range(NT):
                                nc.tensor.matmul(
                                    pt[:, t * 512:(t + 1) * 512],
                                    wmod[:C, b * HID:(b + 1) * HID],
                                    x_in[:C, c0 + t * 512:c0 + (t + 1) * 512],
                                    start=True, stop=True)
                            # epilogue: out = prelu(psum + gcb, 0.2)
                            if last:
                                ost = xst.tile([128, GROUP], BF16, tag="xout")
                                o_full = ost[:]
                            else:
                                o_full = x_out[:, c0:c0 + GROUP]
                            gi = 0 if (gcnt * ACT_SHARE) % ACT_DEN < ACT_SHARE \
                                else 1
                            gcnt += 1
                            if gi == 0:
                                nc.scalar.activation(
                                    o_full, pt[:],
                                    mybir.ActivationFunctionType.Prelu,
                                    bias=gc, alpha=0.2)
                            else:
                                nc.vector._custom_dve(
                                    SCALE_BIAS_LRELU,
                                    out=o_full, in0=pt[:],
                                    s0=ones[:, 0:1], s1=gc,
                                    imm2=0.2)
                            if last:
                                nc.gpsimd.dma_start(
                                    y_d[b, :, col0 + g * GROUP:
                                        col0 + (g + 1) * GROUP],
                                    ost[:])
                for s in pair:
                    del bufs[s]

    nc.compile()
    return nc


def _prep_inputs(pre_point_features, points_encoding, wp,
                 aff_w_in, aff_b_in, conv_w_in, conv_b_in,
                 aff_w, aff_b, conv_w, conv_b):
    """Host-side prep: layout of x + the (tiny) style path fully folded into
    per-(layer,batch) modulated-demodulated weights."""
    import ml_dtypes
    x = np.ascontiguousarray(np.asarray(points_encoding, np.float32)
                             .reshape(B, CIN, HWTOT)
                             .astype(ml_dtypes.bfloat16))
    wp = np.asarray(wp, np.float32)

    def fold(wl, aw, ab, cw):
        # wl [B,WDIM]; aw [C,WDIM]; ab [C]; cw [O,C] -> wmod [C, B*O]
        s = wl @ aw.T * INV_SQRT_WDIM + ab                      # [B, C]
        d = 1.0 / np.sqrt((s * s) @ (cw * cw).T + EPS)          # [B, O]
        wmod = (cw.T[None, :, :] * s[:, :, None] * d[:, None, :]
                * SQRT2)                                        # [B, C, O]
        return np.ascontiguousarray(
            wmod.transpose(1, 0, 2).reshape(wmod.shape[1], B * cw.shape[0]))

    aff_w_in = np.asarray(aff_w_in, np.float32)
    aff_b_in = np.asarray(aff_b_in, np.float32)
    conv_w_in = np.asarray(conv_w_in, np.float32)
    aff_w = np.asarray(aff_w, np.float32)
    aff_b = np.asarray(aff_b, np.float32)
    conv_w = np.asarray(conv_w, np.float32)

    wm0 = fold(wp[:, 0], aff_w_in, aff_b_in, conv_w_in)          # [CIN, B*HID]
    wm0 = wm0.astype(ml_dtypes.bfloat16)
    wmr = np.stack([fold(wp[:, 1 + i], aff_w[i], aff_b[i], conv_w[i])
                    for i in range(NB - 1)], axis=1)             # [HID,NB-1,B*HID]
    wmr = np.ascontiguousarray(wmr)

    gcb = np.empty((HID, NB), np.float32)
    gcb[:, 0] = SQRT2 * np.asarray(conv_b_in, np.float32)
    gcb[:, 1:] = SQRT2 * np.asarray(conv_b, np.float32).T

    shared = dict(wm0=wm0, wmr=wmr, gcb=gcb)
    in_maps = []
    for c in range(N_CORES):
        m = dict(shared)
        m["x"] = np.ascontiguousarray(x[:, :, c * SHARD:(c + 1) * SHARD])
        in_maps.append(m)
    return in_maps


def kernel(trace=False, **inputs):
    global _COMPILED
    if _COMPILED is None:
        _COMPILED = _build()
    nc = _COMPILED
    in_maps = _prep_inputs(**inputs)
    res = run_bass_kernel_spmd(nc, in_maps, core_ids=list(range(N_CORES)),
                               trace=trace)
    parts = [np.asarray(res.results[c]["y"]).astype(np.float32)
             for c in range(N_CORES)]
    out = np.concatenate(parts, axis=2).reshape(B, HID, H, W)
    if trace:
        kernel.last_result = res
    return out
